# revision 71
# baseline (speedup 1.0000x reference)
"""MixerDiffAttention Trainium2 kernel (8-core tensor-parallel over head pairs).

Reference math (B=2, T=2048, D=2048, 16 heads x 256, diff-attention):
  q = x @ Wq.T; k = x @ Wk.T; v = x @ Wv.T   (v as 8 heads x 512)
  q,k: per-head rms_norm then rotary(dim=256)
  split heads into two streams of 8; y = attn1(q1,k1,v) - lam*attn2(q2,k2,v)

Sharding: head-pair i (heads i and i+8 of q/k, v-head i) -> core i.
x replicated; host pre-quantizes x and the W shards to fp8e4 plus fp8e4
residuals, so projections run as DoubleRow (0.5 cyc/row, 256-contraction)
matmul chains:
  q,k,v: x8@W8 + xr@W8 + x8@Wr   (3 chains; ~0.2% effective error,
  0.75x the PE cycles of a bf16 projection)
W is pre-scaled by 64 on host; q/k absorb it in rms_norm, v divides at the
PSUM->SBUF copy. Attention operands (kT, qT, p, v) are fp16: full-rate PE
matmuls with ~0.05% quantization noise. p = exp(score - 5.5) keeps fp16 in
range; the e^-5.5 cancels in the final division by l1.

Key structure (vs the obvious two-stream formulation): the two attention
streams are merged in p-space before the p@v matmul --
  ptilde = p1 - p2 * (lam*l1/l2)[q]     (per-query scale, DVE)
  y      = (ptilde @ v) / l1            (per-partition scale, ACT)
which halves the p@v PE work and removes the second normalize+subtract.
The per-query scale row cb = l1/l2 is transposed on PE, broadcast across
partitions with a K=1 matmul against a lam-valued row (folding lam in
for free), and consumed by the merges from SBUF (first merge reads the
PSUM copy directly to skip one latency hop).

Per-core pipeline (token-major phase -> d-major attention):
  proj (PE fp8-DR) -> rms stats (ACT Square+accum from PSUM) -> rotary
  (DVE f16, reading PSUM directly) -> r = exp(-.5 ln(ms+eps)) (ACT) ->
  normalize (ACT scale, f16) -> kT/qT via XBAR DMA transpose (zero PE).
  scores ST[kt,qt] = kT.T @ qT (PE f16), two key tiles paired per PSUM
  bank -> causal -1e30 add (DVE, diag tiles) -> exp-5.5 per pair
  (ACT -> f16 p) -> l += p.T @ ones (PE) for both streams -> cb chain
  (DVE+PE) -> ptilde merges (DVE) -> out += pt.T @ v (PE, one chunk
  later) -> y = out * (1/l1) (ACT, PSUM->SBUF f16) -> DMA out.

Scheduling: everything is software-pipelined around the in-order engine
queues. The DMA engines serialize transfers, so the startup load order
matches PE consumption (k weights + first tokens in fine slices, then
v weights; v projections lag k by 2 tiles). Query chunks are processed
HIGH-to-LOW so the un-overlappable trailing p@v is the 1-pair chunk,
with projections prefetched two chunks ahead and drained as PE filler
inside the exp-paced score phase; the p@v matmuls of chunk i run as
filler inside chunk i+1's score phase, which hides the serial
l->cb->ptilde chain entirely; batch 1's phase A overlaps batch 0's
attention tail (its x reload is ordered to match the reversed reader
order).
"""
import math
from contextlib import ExitStack

import numpy as np
import ml_dtypes

import concourse.bass as bass
import concourse.bacc as bacc
import concourse.tile as tile
import concourse.mybir as mybir
from concourse.bass_utils import run_bass_kernel_spmd

F32 = mybir.dt.float32
F16 = mybir.dt.float16
FP8 = mybir.dt.float8e4
AF = mybir.ActivationFunctionType
ALU = mybir.AluOpType

NP_FP8 = ml_dtypes.float8_e4m3fn

B = 2
D = 2048
N_HEADS = 16
HEAD_DIM = 256
OF = 512                      # per-core q/k/v feature width (2 heads x 256)
NG = D // 256                 # DoubleRow contraction-pair groups (8)
WS = 64.0                     # host weight prescale
LAMBDA_INIT = 0.8 - 0.6 * math.exp(-0.3 * 0)
EPS = float(np.finfo(np.float32).eps)
NEG = -1.0e30
PBIAS = -5.5                  # exp bias so p fits fp16 (cancels in o/l)


def q8(x: np.ndarray) -> np.ndarray:
    """Quantize to TRN e4m3 (clip to +-240, RNE)."""
    return np.clip(np.asarray(x, np.float32), -240.0, 240.0).astype(NP_FP8)


_TABLES_PATCHED = False


def _patch_act_tables():
    """Force every activation into natural_log_exp_and_others so the kernel
    needs exactly one ACT table load."""
    global _TABLES_PATCHED
    if _TABLES_PATCHED:
        return
    import concourse.hw_specs as hw_specs
    mine = {AF.Exp, AF.Ln, AF.Square, AF.Copy, AF.Identity, AF.Rsqrt}
    orig = hw_specs.get_activation_tables

    def patched(arch):
        out = {}
        for name, funcs in orig(arch).items():
            out[name] = funcs if name == "natural_log_exp_and_others" \
                else (funcs - mine)
        return out

    bacc.get_activation_tables = patched
    _TABLES_PATCHED = True


def build_nc(T: int = 2048):
    """Build the single-core SPMD program for per-batch token count T."""
    NT = T // 128            # token tiles per batch
    NQC = T // 256           # query chunks per batch
    _patch_act_tables()
    nc = bacc.Bacc("TRN2", target_bir_lowering=False, debug=False)

    x8d = nc.dram_tensor("x8d", [D, B * T], FP8, kind="ExternalInput").ap()
    xrd = nc.dram_tensor("xrd", [D, B * T], FP8, kind="ExternalInput").ap()
    wd = {}
    for nm in ("wq8", "wqr", "wk8", "wkr", "wv8", "wvr"):
        wd[nm] = nc.dram_tensor(nm, [D, OF], FP8, kind="ExternalInput").ap()
    cosd = nc.dram_tensor("cosd", [128, NT * 128], F16, kind="ExternalInput").ap()
    sind = nc.dram_tensor("sind", [128, NT * 128], F16, kind="ExternalInput").ap()
    identd = nc.dram_tensor("identd", [128, 128], F16, kind="ExternalInput").ap()
    onesd = nc.dram_tensor("onesd", [128, 1], F16, kind="ExternalInput").ap()
    maskd = nc.dram_tensor("maskd", [128, 384], F32, kind="ExternalInput").ap()
    lamd = nc.dram_tensor("lamd", [128, 1], F32, kind="ExternalInput").ap()
    lamrowd = nc.dram_tensor("lamrowd", [1, 128], F16, kind="ExternalInput").ap()
    out = nc.dram_tensor("out", [B, T, OF], F16, kind="ExternalOutput").ap()

    # [128, (g two), t]: contraction chunk c = 2g+two, row p -> d = c*128+p
    x8_r = x8d.rearrange("(g two p) t -> p g two t", p=128, two=2)
    xr_r = xrd.rearrange("(g two p) t -> p g two t", p=128, two=2)
    w_r = {nm: ap.rearrange("(g two p) n -> p g two n", p=128, two=2)
           for nm, ap in wd.items()}

    with tile.TileContext(nc) as tc, ExitStack() as ctx:
        # --- constant pools (loaded once) ---
        cpool = ctx.enter_context(tc.tile_pool(name="consts", bufs=1))
        cos_t = cpool.tile([128, NT, 128], F16, tag="cos")
        sin_t = cpool.tile([128, NT, 128], F16, tag="sin")
        id_t = cpool.tile([128, 128], F16, tag="ident")
        ones_t = cpool.tile([128, 1], F16, tag="ones")
        lam_row = cpool.tile([1, 128], F16, tag="lamrow")
        mask_t = cpool.tile([128, 384], F32, tag="mask")
        eps_t = cpool.tile([128, 1], F32, tag="eps")
        ln16_t = cpool.tile([128, 1], F32, tag="ln16")
        pbias_t = cpool.tile([128, 1], F32, tag="pbias")
        nc.vector.memset(eps_t[:], EPS)
        nc.vector.memset(ln16_t[:], -math.log(16.0))
        nc.vector.memset(pbias_t[:], PBIAS)

        cos_r = cosd.rearrange("p (n f) -> p n f", f=128)
        sin_r = sind.rearrange("p (n f) -> p n f", f=128)

        def load_consts_early(nt0, nt1):
            # the first tiles' rotary tables + the small constants
            nc.sync.dma_start(cos_t[:, nt0:nt1], cos_r[:, nt0:nt1])
            nc.sync.dma_start(sin_t[:, nt0:nt1], sin_r[:, nt0:nt1])

        def load_consts_small():
            nc.sync.dma_start(id_t[:], identd)
            nc.sync.dma_start(ones_t[:], onesd)
            nc.sync.dma_start(mask_t[:], maskd)
            nc.sync.dma_start(lam_row[:], lamrowd)

        # --- weights: all six fp8 shards resident for the whole kernel ---
        wpool = ctx.enter_context(tc.tile_pool(name="weights", bufs=1))
        wt = {}
        for nm in ("wq8", "wqr", "wk8", "wkr", "wv8", "wvr"):
            wt[nm] = wpool.tile([128, NG, 2, OF], FP8, tag=nm, name=nm)

        # --- per-batch resident x (fp8 + residual) ---
        xpool = ctx.enter_context(tc.tile_pool(name="x", bufs=1))
        x8sb = xpool.tile([128, NG, 2, T], FP8, tag="x8sb", name="x8sb")
        xrsb = xpool.tile([128, NG, 2, T], FP8, tag="xrsb", name="xrsb")

        # --- per-batch key/value caches ---
        kvpool = ctx.enter_context(tc.tile_pool(name="kv", bufs=1))
        # --- streaming pools ---
        tpool = ctx.enter_context(tc.tile_pool(name="t", bufs=2))
        qtpool = ctx.enter_context(tc.tile_pool(name="qt", bufs=3))
        # pp tiles for the whole query chunk (both streams) stay live until
        # the ptilde merge: 2*(NQC pairs) + in-flight
        ppool = ctx.enter_context(tc.tile_pool(name="p", bufs=2 * NQC + 2))
        ptpool = ctx.enter_context(tc.tile_pool(name="pt", bufs=2))
        spool = ctx.enter_context(tc.tile_pool(name="stats", bufs=6))
        cbpool = ctx.enter_context(tc.tile_pool(name="cb", bufs=2))
        ypool = ctx.enter_context(tc.tile_pool(name="y", bufs=2))
        # --- PSUM pools: big 2 + out 2 + l 1 + (st|cb shared) 3 = 8 banks ---
        bigps = ctx.enter_context(tc.tile_pool(name="bigps", bufs=2, space="PSUM"))
        outps = ctx.enter_context(tc.tile_pool(name="outps", bufs=2, space="PSUM"))
        lps = ctx.enter_context(tc.tile_pool(name="lps", bufs=1, space="PSUM"))
        strps = ctx.enter_context(tc.tile_pool(name="strps", bufs=3, space="PSUM"))

        def proj(t0, chains):
            """PSUM [tok 128, OF] via DoubleRow accumulation chains.

            chains: list of (x_tile, w_tile); each contributes NG DR matmuls.
            """
            ps = bigps.tile([128, OF], F32, tag="big", name="projps")
            n = len(chains) * NG
            i = 0
            for xt, wtile in chains:
                for g in range(NG):
                    nc.tensor.matmul(
                        ps[:], xt[:, g, :, t0:t0 + 128], wtile[:, g],
                        start=(i == 0), stop=(i == n - 1),
                        perf_mode=mybir.MatmulPerfMode.DoubleRow)
                    i += 1
            return ps

        def qk_rot(ps, tt, is_q):
            """rms-stats + rotary + normalize [tok,512] psum -> yn f16 SBUF.

            ACT/DVE only (no PE). Rotary multiplies read the proj PSUM
            directly (f16 products); the bank frees after the two DVE mults.
            r = Rsqrt fuses the whole rms scale: for q,
            rsqrt(ss/256+eps)/16 == rsqrt(ss + 256 eps).
            """
            # sum of squares per head: ACT Square + free-dim accumulate
            tsq = tpool.tile([128, OF], F16, tag="tsq")
            ss = spool.tile([128, 2], F32, tag="ss")
            for h in range(2):
                nc.scalar.activation(
                    tsq[:, h * 256:(h + 1) * 256],
                    ps[:, h * 256:(h + 1) * 256],
                    AF.Square, accum_out=ss[:, h:h + 1])
            # rotary multiplies straight from PSUM, f16 out
            cos_b = cos_t[:, tt:tt + 1, :].broadcast_to([128, 4, 128])
            sin_b = sin_t[:, tt:tt + 1, :].broadcast_to([128, 4, 128])
            t1t = tpool.tile([128, OF], F16, tag="t1")
            t2t = tpool.tile([128, OF], F16, tag="t2")
            ps4 = ps.rearrange("p (a f) -> p a f", f=128)
            t1 = t1t.rearrange("p (a f) -> p a f", f=128)
            t2 = t2t.rearrange("p (a f) -> p a f", f=128)
            nc.vector.tensor_tensor(t1[:], ps4[:], cos_b, ALU.mult)
            nc.vector.tensor_tensor(t2[:], ps4[:], sin_b, ALU.mult)
            # r = rsqrt(mean + eps) = exp(-0.5*ln(mean+eps)); q also /16
            lg = spool.tile([128, 2], F32, tag="lg")
            nc.scalar.activation(lg[:], ss[:], AF.Ln, scale=1.0 / 256.0,
                                 bias=eps_t[:, 0:1])
            r = spool.tile([128, 2], F32, tag="r")
            nc.scalar.activation(r[:], lg[:], AF.Exp, scale=-0.5,
                                 bias=(ln16_t[:, 0:1] if is_q else 0.0))
            # rotary combine (f16, 2x DVE) then normalize
            ytt = tpool.tile([128, OF], F16, tag="yt")
            yv = ytt.rearrange("p (h u f) -> p h u f", h=2, f=128)
            t1v = t1t.rearrange("p (h u f) -> p h u f", h=2, f=128)
            t2v = t2t.rearrange("p (h u f) -> p h u f", h=2, f=128)
            nc.vector.tensor_tensor(yv[:, :, 0], t1v[:, :, 0], t2v[:, :, 1], ALU.add)
            nc.vector.tensor_tensor(yv[:, :, 1], t1v[:, :, 1], t2v[:, :, 0],
                                    ALU.subtract)
            ynt = tpool.tile([128, OF], F16, tag="yn")
            yn = ynt[:]
            for h in range(2):
                nc.scalar.mul(yn[:, h * 256:(h + 1) * 256],
                              ytt[:, h * 256:(h + 1) * 256], r[:, h:h + 1])
            return ynt

        def do_transpose(yn, dst, dst_col):
            """DMA-transpose yn [tok,512] f16 into dst[:, :, dst_col:+128].

            dst[p, c, t] = yn[t, c*128 + p] (XBAR tile transpose) — runs on
            the DMA engines, so it never blocks the PE queue. Latency is
            hidden by the one-chunk-ahead pipelining.
            """
            nc.sync.dma_start_transpose(dst[:, :, dst_col:dst_col + 128],
                                        yn[:])

        def dma_x(c0, c1, bb, g=None):
            dsl = slice(c0, c1)
            ssl = slice(bb * T + c0, bb * T + c1)
            if g is None:
                nc.sync.dma_start(x8sb[:, :, :, dsl], x8_r[:, :, :, ssl])
                nc.sync.dma_start(xrsb[:, :, :, dsl], xr_r[:, :, :, ssl])
            else:
                nc.sync.dma_start(x8sb[:, g, :, dsl], x8_r[:, g, :, ssl])

        def dma_xr(c0, c1, bb, g):
            ssl = slice(bb * T + c0, bb * T + c1)
            nc.sync.dma_start(xrsb[:, g, :, c0:c1], xr_r[:, g, :, ssl])

        def dma_w(nm, h):
            hs = slice(h * 2, h * 2 + 2)
            nc.sync.dma_start(wt[nm][:, hs], w_r[nm][:, hs])

        for b in range(B):
            # DMA transfers serialize on the DMA engines, so the load order
            # is arranged to match PE consumption: k weights + the first x
            # tokens in fine slices, rotary tables for the first tiles,
            # the k-residual chain inputs, then v weights and the remaining
            # x in >=512B-line chunks (lines under 512B pay 2x DMA latency).
            if T < 2048:
                dma_x(0, T, b)
                if b == 0:
                    for nm in ("wk8", "wkr", "wv8", "wvr", "wq8", "wqr"):
                        nc.sync.dma_start(wt[nm][:], w_r[nm])
                    load_consts_early(0, NT)
                    load_consts_small()
            elif b == 0:
                for h in range(4):
                    dma_w("wk8", h)
                    dma_x(0, 256, b, g=2 * h)
                    dma_x(0, 256, b, g=2 * h + 1)
                load_consts_early(0, 4)
                load_consts_small()
                for h in range(4):
                    dma_w("wkr", h)
                    dma_xr(0, 256, b, 2 * h)
                    dma_xr(0, 256, b, 2 * h + 1)
                dma_x(256, 768, b)
                for h in range(4):
                    dma_w("wv8", h)
                dma_x(768, 1280, b)
                for h in range(4):
                    dma_w("wvr", h)
                load_consts_early(4, NT)
                dma_x(1280, 1792, b)
                nc.sync.dma_start(wt["wq8"][:], w_r["wq8"])
                dma_x(1792, T, b)
                nc.sync.dma_start(wt["wqr"][:], w_r["wqr"])
            else:
                # batch 0's query chunks run high-to-low, so its readers of
                # the HIGH token range finish first: reload those first
                for c0 in range(T - 512, -1, -512):
                    dma_x(c0, c0 + 512, b)

            kT = kvpool.tile([128, 4, T], F16, tag="kT")
            vsb = kvpool.tile([128, NT, OF], F16, tag="v")

            # --- Phase A: keys and values (kT built by DMA transpose).
            # v-projections lag the k-projections by VLAG tiles: the v
            # weights arrive later in the startup DMA stream, and the
            # v-only tail is a convenient PE-heavy window to hide the
            # phase-B q-chunk priming's DVE work under. ---
            def do_k(tt):
                kps = proj(tt * 128, [(x8sb, wt["wk8"]), (xrsb, wt["wk8"]),
                                      (x8sb, wt["wkr"])])
                yn = qk_rot(kps, tt, False)
                do_transpose(yn, kT, tt * 128)

            def do_v(tt):
                vps = proj(tt * 128, [(x8sb, wt["wv8"]), (xrsb, wt["wv8"]),
                                      (x8sb, wt["wvr"])])
                nc.vector.tensor_scalar_mul(vsb[:, tt], vps[:], 1.0 / WS)

            # v lags k by VLAG tiles: the v weights arrive after the k
            # weights and first x chunks in the serialized DMA stream
            VLAG = min(4, NT // 2)
            aseq = []
            for tt in range(NT):
                aseq.append(('k', tt))
                if tt >= VLAG:
                    aseq.append(('v', tt - VLAG))
            for tt in range(NT - VLAG, NT):
                aseq.append(('v', tt))

            # --- Phase B: queries + attention, q-chunks prefetched TWO qc
            # ahead. The prefetched chunk's PE projections are emitted as
            # single-matmul thunks and drained as filler wherever PE would
            # otherwise wait (between score pairs while exp catches up, and
            # under the DVE cb-chain); its DVE rotary is emitted after the
            # ptilde merges so the merges aren't stuck behind it in the
            # in-order DVE queue.
            def q_proj_thunks(qc):
                """Allocate PSUM + return (ps_tiles, [thunk emitting 1 mm])."""
                pss = []
                thunks = []
                for u in range(2):
                    ps = bigps.tile([128, OF], F32, tag="big",
                                    name=f"qps{qc}_{u}")
                    pss.append(ps)
                    t0 = (qc * 2 + u) * 128
                    chain = [(x8sb, wt["wq8"]), (xrsb, wt["wq8"]),
                             (x8sb, wt["wqr"])]
                    n = len(chain) * NG

                    def mk(ps_, t0_, xt_, wtile_, g_, i_):
                        def emit():
                            nc.tensor.matmul(
                                ps_[:], xt_[:, g_, :, t0_:t0_ + 128],
                                wtile_[:, g_],
                                start=(i_ == 0), stop=(i_ == n - 1),
                                perf_mode=mybir.MatmulPerfMode.DoubleRow)
                        return emit
                    i = 0
                    for xt, wtile in chain:
                        for g in range(NG):
                            thunks.append(mk(ps, t0, xt, wtile, g, i))
                            i += 1
                return pss, thunks

            def q_rot_part(qc, qpss):
                qT = qtpool.tile([128, 4, 256], F16, tag="qT", name=f"qT{qc}")
                for u in range(2):
                    yn = qk_rot(qpss[u], qc * 2 + u, True)
                    do_transpose(yn, qT, u * 128)
                return qT

            # prime chunks 0 and 1 (their rotary DVE work hides under the
            # v-projection tail of phase A)
            def q_chunk_now(qc):
                pss, thunks = q_proj_thunks(qc)
                for th in thunks:
                    th()
                return q_rot_part(qc, pss)

            # batch 1's query chunks run in REVERSE order: the trailing,
            # unoverlappable p@v at the very end of the kernel is then the
            # 1-pair qc=0 instead of the full-width last chunk
            qorder = list(range(NQC))
            if NQC > 1:
                qorder = qorder[::-1]

            qT_cur = qT_next = None
            tail = max(len(aseq) - 4, 0)
            for idx, (kind, tt) in enumerate(aseq):
                if idx == tail:
                    qT_cur = q_chunk_now(qorder[0])
                if idx == tail + 2 and NQC > 1:
                    qT_next = q_chunk_now(qorder[1])
                (do_k if kind == 'k' else do_v)(tt)
            if qT_cur is None:
                qT_cur = q_chunk_now(qorder[0])
            if qT_next is None and NQC > 1:
                qT_next = q_chunk_now(qorder[1])
            # Two-stage software pipeline over query chunks: iteration qc
            # emits S(qc) (scores+exp+l), the cb chain and ptilde merges for
            # qc, but the p@v matmuls for qc-1 — interleaved into S(qc) as
            # PE filler. The ptilde tensors of qc-1 were finished while
            # S(qc-1)'s tail ran, so pv(qc-1) is never blocked, and the cb
            # chain of qc hides under pv(qc-1)'s leftovers + projections.
            prev = None          # (pts, linv, qc) of the previous iteration

            def emit_pv(state, op_, j):
                pts_, _linv, qcp = state
                nktp = 2 * (qcp + 1)
                pt = pts_[j]
                for sl in range(2):
                    kt = 2 * j + sl
                    for u in range(2):
                        if kt == nktp - 1 and u == 0:
                            continue
                        lastu = nktp - 1 if u == 1 else nktp - 2
                        nc.tensor.matmul(
                            op_[u][:], pt[:, sl, u * 128:(u + 1) * 128],
                            vsb[:, kt], start=(kt == 0), stop=(kt == lastu))

            def emit_final(state, op_):
                _pts, linv_, qcp = state
                for u in range(2):
                    yf = ypool.tile([128, OF], F16, tag="yf")
                    nc.scalar.mul(yf[:], op_[u][:], linv_[:, 0, u:u + 1])
                    t0 = qcp * 256 + u * 128
                    nc.sync.dma_start(out[b, t0:t0 + 128, :], yf[:])

            for i_qc in range(NQC):
                qc = qorder[i_qc]
                qT = qT_cur

                # the chunk prefetched two ahead: its projection matmuls are
                # drained as PE filler through this iteration
                if i_qc + 2 < NQC:
                    pf_pss, pf_thunks = q_proj_thunks(qorder[i_qc + 2])
                else:
                    pf_pss, pf_thunks = None, []

                op = [outps.tile([128, OF], F32, tag="o", name=f"op{u_}")
                      for u_ in range(2)] if prev is not None else None
                pvq = list(range(prev[2] + 1)) if prev is not None else []

                def filler(k, tail=False):
                    for _ in range(k):
                        if pvq:
                            emit_pv(prev, op, pvq.pop(0))
                        elif pf_thunks:
                            pf_thunks.pop(0)()

                nkt = 2 * (qc + 1)
                npair = nkt // 2
                # l accumulators for both streams / both query halves
                lp = lps.tile([128, 2, 2, NT], F32, tag="l")
                pps = [[None] * npair, [None] * npair]

                def emit_pair(s, j):
                    # two key tiles share one PSUM bank: slot 0's
                    # start=True lazily zeroes the whole bank, slot 1
                    # accumulates onto the pending zeros.
                    stp = strps.tile([128, 2, 256], F32, tag="str",
                                     name="stpair")
                    i = 0
                    for sl in range(2):
                        kt = 2 * j + sl
                        qcols = slice(128, 256) if kt == nkt - 1 \
                            else slice(0, 256)
                        for c2 in range(2):
                            nc.tensor.matmul(
                                stp[:, sl, qcols],
                                kT[:, 2 * s + c2, kt * 128:(kt + 1) * 128],
                                qT[:, 2 * s + c2, qcols],
                                start=(i == 0), stop=(i == 3),
                                skip_group_check=True)
                            i += 1
                    if j == qc:
                        # diagonal pair: triangular mask per u-half
                        for sl in range(2):
                            mc = slice(0, 128) if sl == 0 \
                                else slice(128, 256)
                            nc.vector.tensor_tensor(
                                stp[:, sl, mc], stp[:, sl, mc],
                                mask_t[:, 128:256], ALU.add)
                    return stp

                def emit_exp(s, j, stp):
                    pp = ppool.tile([128, 2, 256], F16, tag="p")
                    with tc.high_priority(offset=100):
                        if j == qc:   # diag pair: slot 1 only u1 cols
                            nc.scalar.activation(pp[:, 0], stp[:, 0],
                                                 AF.Exp,
                                                 bias=pbias_t[:, 0:1])
                            nc.scalar.activation(pp[:, 1, 128:256],
                                                 stp[:, 1, 128:256],
                                                 AF.Exp,
                                                 bias=pbias_t[:, 0:1])
                        else:
                            nc.scalar.activation(pp[:], stp[:], AF.Exp,
                                                 bias=pbias_t[:, 0:1])
                    return pp

                # --- S phase: scores + exp + l for both streams, with
                # prefetch-projection matmuls as filler (exp throughput is
                # the S-phase bottleneck; the filler soaks PE's wait).
                # Stream 0's l reduction is emitted mid-phase so it hides
                # under stream 1's score matmuls. ---
                lsum = spool.tile([128, 2, 2], F32, tag="lsum")
                linv = spool.tile([128, 2, 2], F32, tag="linv")
                jorder = list(range(npair))
                for s in range(2):
                    pairq = [(j_, emit_pair(s, j_))
                             for j_ in jorder[:min(2, npair)]]
                    for i_ in range(npair):
                        j, stp = pairq.pop(0)
                        pp = emit_exp(s, j, stp)
                        pps[s][j] = pp
                        if i_ + 2 < npair:
                            j2 = jorder[i_ + 2]
                            pairq.append((j2, emit_pair(s, j2)))
                        filler(1)
                        for sl in range(2):
                            kt = 2 * j + sl
                            for u in range(2):
                                if kt == nkt - 1 and u == 0:
                                    continue
                                nc.tensor.matmul(
                                    lp[:, s, u, kt:kt + 1],
                                    pp[:, sl, u * 128:(u + 1) * 128],
                                    ones_t[:], start=True, stop=True)
                    nc.vector.reduce_sum(lsum[:, s, 0:1],
                                         lp[:, s, 0, 0:max(nkt - 1, 1)],
                                         axis=mybir.AxisListType.X)
                    nc.vector.reduce_sum(lsum[:, s, 1:2],
                                         lp[:, s, 1, 0:nkt],
                                         axis=mybir.AxisListType.X)
                    nc.vector.reciprocal(linv[:, s], lsum[:, s])
                # cb2 = l1/l2 (lam is folded into the broadcast matmul)
                cb16 = spool.tile([128, 2], F16, tag="cb16")
                nc.vector.tensor_tensor(cb16[:], lsum[:, 0], linv[:, 1],
                                        ALU.mult)

                # drain remaining pv chunks + projections over the serial
                # lsum/cb chain latency
                filler(len(pvq) + len(pf_thunks))

                # transpose each cb16 column to its own partition-0 row
                # (engines can only address partitions 0/32/64)
                cbT = [strps.tile([1, 128], F16, tag="str", name=f"cbT{u_}")
                       for u_ in range(2)]
                for u in range(2):
                    nc.tensor.transpose(cbT[u][:], cb16[:, u:u + 1], id_t[:])
                cbrow = [cbpool.tile([1, 128], F16, tag=f"cbrow{u_}",
                                     name=f"cbrow{u_}")
                         for u_ in range(2)]
                for u in range(2):
                    nc.vector.tensor_copy(cbrow[u][:], cbT[u][:])
                cbbps = strps.tile([128, 2, 128], F32, tag="str", name="cbb")
                for u in range(2):
                    nc.tensor.matmul(cbbps[:, u], lam_row[:], cbrow[u][:],
                                     start=True, stop=True,
                                     skip_group_check=True)
                cbb = cbpool.tile([128, 1, 256], F16, tag="cbbs")
                nc.vector.tensor_copy(cbb[:, 0], cbbps[:])
                cbb_b = cbb[:].broadcast_to([128, 2, 256])

                # --- ptilde merges for THIS qc (consumed next iteration) ---
                pts = []
                cbbps_b = cbbps[:].rearrange("p u f -> p (u f)") \
                    .rearrange("p (one f) -> p one f", one=1) \
                    .broadcast_to([128, 2, 256])
                for j in range(npair):
                    tmp = ptpool.tile([128, 2, 256], F16, tag="pt", bufs=2)
                    # the first merge reads the broadcast row straight from
                    # PSUM (slower op, but skips waiting for the SBUF copy)
                    nc.vector.tensor_tensor(tmp[:], pps[1][j][:],
                                            cbbps_b if j == 0 else cbb_b,
                                            ALU.mult)
                    pt = ptpool.tile([128, 2, 256], F16, tag="pt2",
                                     bufs=NQC + 2)
                    nc.vector.tensor_tensor(pt[:], pps[0][j][:], tmp[:],
                                            ALU.subtract)
                    pts.append(pt)

                # drain the rest of pv(qc-1) and close it out
                filler(len(pvq) + len(pf_thunks))
                if prev is not None:
                    emit_final(prev, op)

                # the prefetched chunk's DVE rotary + transposes, behind
                # the ptilde merges in the DVE queue
                qT_cur = qT_next
                if pf_pss is not None:
                    qT_next = q_rot_part(qorder[i_qc + 2], pf_pss)

                prev = (pts, linv, qc)

            # trailing p@v for the last query chunk of this batch
            op = [outps.tile([128, OF], F32, tag="o", name=f"opt{u_}")
                  for u_ in range(2)]
            for j in range(prev[2] + 1):
                emit_pv(prev, op, j)
            emit_final(prev, op)
    nc.compile()
    return nc


def make_in_maps(x, Wq, Wk, Wv, lam, T):
    """Host-side sharding + fp8/f16 layout prep. Returns list of 8 in_maps."""
    NT = T // 128
    xf = np.ascontiguousarray(x.reshape(B * T, D).T).astype(np.float32)
    x8 = q8(xf)
    xr = q8(xf - x8.astype(np.float32))
    t = np.arange(T, dtype=np.float64)
    inv = 1.0 / (10000.0 ** (np.arange(0, HEAD_DIM, 2, dtype=np.float64)
                             / HEAD_DIM))
    fr = np.outer(t, inv)                                    # [T, 128]
    cos = np.cos(fr).astype(np.float32)
    sin = np.sin(fr).astype(np.float32)
    # [128, NT*128]: row p, col tt*128+f  ->  cos[tt*128+p, f]
    cos_sb = np.ascontiguousarray(
        cos.reshape(NT, 128, 128).transpose(1, 0, 2).reshape(128, NT * 128)
    ).astype(np.float16)
    sin_sb = np.ascontiguousarray(
        sin.reshape(NT, 128, 128).transpose(1, 0, 2).reshape(128, NT * 128)
    ).astype(np.float16)
    ident = np.eye(128, dtype=np.float16)
    ones1 = np.ones((128, 1), np.float16)
    ii = np.arange(128).reshape(128, 1)
    mm_ = np.arange(384).reshape(1, 384) - 128
    maskneg = np.where(mm_ >= ii, 0.0, NEG).astype(np.float32)
    lam_np = np.full((128, 1), lam, np.float32)
    lam_row_np = np.full((1, 128), lam, np.float16)

    common = {"x8d": x8, "xrd": xr, "cosd": cos_sb, "sind": sin_sb,
              "identd": ident, "onesd": ones1, "maskd": maskneg,
              "lamd": lam_np, "lamrowd": lam_row_np}
    in_maps = []
    for i in range(8):
        def shards(W, half):
            sh = np.concatenate(
                [W[i * 256:(i + 1) * 256], W[(i + 8) * 256:(i + 9) * 256]], 0
            ) if half else W[i * 512:(i + 1) * 512]
            wT = np.ascontiguousarray(sh.T).astype(np.float32) * WS
            w8 = q8(wT)
            wr = q8(wT - w8.astype(np.float32))
            return w8, wr
        m = dict(common)
        m["wq8"], m["wqr"] = shards(np.asarray(Wq), True)
        m["wk8"], m["wkr"] = shards(np.asarray(Wk), True)
        m["wv8"], m["wvr"] = shards(np.asarray(Wv), False)
        in_maps.append(m)
    return in_maps


_NC_CACHE: dict = {}


def run_cores(x, Wq, Wk, Wv, lambda_q1, lambda_k1, lambda_q2, lambda_k2,
              T=2048, **spmd_kwargs):
    lam1 = np.exp(np.float32(np.dot(lambda_q1.astype(np.float32),
                                    lambda_k1.astype(np.float32))))
    lam2 = np.exp(np.float32(np.dot(lambda_q2.astype(np.float32),
                                    lambda_k2.astype(np.float32))))
    lam = np.float32(lam1 - lam2 + np.float32(LAMBDA_INIT))
    if T not in _NC_CACHE:
        _NC_CACHE[T] = build_nc(T)
    nc = _NC_CACHE[T]
    in_maps = make_in_maps(np.asarray(x), np.asarray(Wq), np.asarray(Wk),
                           np.asarray(Wv), lam, T)
    res = run_bass_kernel_spmd(nc, in_maps, core_ids=list(range(8)),
                               **spmd_kwargs)
    shards = [res.results[i]["out"] for i in range(8)]       # [B,T,512] each
    y = np.stack(shards, axis=2).reshape(B, T, N_HEADS * HEAD_DIM)
    return y, res


def kernel(x, Wq, Wk, Wv, lambda_q1, lambda_k1, lambda_q2, lambda_k2):
    y, _ = run_cores(x, Wq, Wk, Wv, lambda_q1, lambda_k1, lambda_q2,
                     lambda_k2, T=x.shape[1])
    return y.astype(np.float32)


# revision 77
# speedup vs baseline: 1.0422x; 1.0422x over previous
"""MixerDiffAttention Trainium2 kernel (8-core tensor-parallel over head pairs).

Reference math (B=2, T=2048, D=2048, 16 heads x 256, diff-attention):
  q = x @ Wq.T; k = x @ Wk.T; v = x @ Wv.T   (v as 8 heads x 512)
  q,k: per-head rms_norm then rotary(dim=256)
  split heads into two streams of 8; y = attn1(q1,k1,v) - lam*attn2(q2,k2,v)

Sharding: head-pair i (heads i and i+8 of q/k, v-head i) -> core i.
x replicated; host pre-quantizes x and the W shards to fp8e4 plus fp8e4
residuals, so projections run as DoubleRow (0.5 cyc/row, 256-contraction)
matmul chains:
  q,k,v: x8@W8 + xr@W8 + x8@Wr   (3 chains; ~0.2% effective error,
  0.75x the PE cycles of a bf16 projection)
W is pre-scaled by 64 on host; q/k absorb it in rms_norm, v divides at the
PSUM->SBUF copy. Attention operands (kT, qT, p, v) are fp16: full-rate PE
matmuls with ~0.05% quantization noise. p = exp(score - 5.5) keeps fp16 in
range; the e^-5.5 cancels in the final division by l1.

Key structure (vs the obvious two-stream formulation): the two attention
streams are merged in p-space before the p@v matmul --
  ptilde = p1 - p2 * (lam*l1/l2)[q]     (per-query scale, DVE)
  y      = (ptilde @ v) / l1            (per-partition scale, ACT)
which halves the p@v PE work and removes the second normalize+subtract.
The per-query scale row cb = l1/l2 is transposed on PE, broadcast across
partitions with a K=1 matmul against a lam-valued row (folding lam in
for free), and consumed by the merges from SBUF (first merge reads the
PSUM copy directly to skip one latency hop).

Per-core pipeline (token-major phase -> d-major attention):
  proj (PE fp8-DR) -> rms stats (ACT Square+accum from PSUM) -> rotary
  (DVE f16, reading PSUM directly) -> r = exp(-.5 ln(ms+eps)) (ACT) ->
  normalize (ACT scale, f16) -> kT/qT via XBAR DMA transpose (zero PE).
  scores ST[kt,qt] = kT.T @ qT (PE f16), two key tiles paired per PSUM
  bank -> causal -1e30 add (DVE, diag tiles) -> exp-5.5 per pair
  (ACT -> f16 p) -> l += p.T @ ones (PE) for both streams -> cb chain
  (DVE+PE) -> ptilde merges (DVE) -> out += pt.T @ v (PE, one chunk
  later) -> y = out * (1/l1) (ACT, PSUM->SBUF f16) -> DMA out.

Scheduling: everything is software-pipelined around the in-order engine
queues. The DMA engines serialize transfers, so the startup load order
matches PE consumption (k weights + first tokens in fine slices, then
v weights; v projections lag k by 2 tiles). Query chunks are processed
HIGH-to-LOW so the un-overlappable trailing p@v is the 1-pair chunk,
with projections prefetched two chunks ahead and drained as PE filler
inside the exp-paced score phase; the p@v matmuls of chunk i run as
filler inside chunk i+1's score phase, which hides the serial
l->cb->ptilde chain entirely; batch 1's phase A overlaps batch 0's
attention tail (its x reload is ordered to match the reversed reader
order).
"""
import math
from contextlib import ExitStack

import numpy as np
import ml_dtypes

import concourse.bass as bass
import concourse.bacc as bacc
import concourse.tile as tile
import concourse.mybir as mybir
from concourse.bass_utils import run_bass_kernel_spmd

F32 = mybir.dt.float32
F16 = mybir.dt.float16
FP8 = mybir.dt.float8e4
AF = mybir.ActivationFunctionType
ALU = mybir.AluOpType

NP_FP8 = ml_dtypes.float8_e4m3fn

B = 2
D = 2048
N_HEADS = 16
HEAD_DIM = 256
OF = 512                      # per-core q/k/v feature width (2 heads x 256)
NG = D // 256                 # DoubleRow contraction-pair groups (8)
WS = 64.0                     # host weight prescale
LAMBDA_INIT = 0.8 - 0.6 * math.exp(-0.3 * 0)
EPS = float(np.finfo(np.float32).eps)
NEG = -1.0e30
PBIAS = -5.5                  # exp bias so p fits fp16 (cancels in o/l)


def q8(x: np.ndarray) -> np.ndarray:
    """Quantize to TRN e4m3 (clip to +-240, RNE)."""
    return np.clip(np.asarray(x, np.float32), -240.0, 240.0).astype(NP_FP8)


_TABLES_PATCHED = False


def _patch_act_tables():
    """Force every activation into natural_log_exp_and_others so the kernel
    needs exactly one ACT table load."""
    global _TABLES_PATCHED
    if _TABLES_PATCHED:
        return
    import concourse.hw_specs as hw_specs
    mine = {AF.Exp, AF.Ln, AF.Square, AF.Copy, AF.Identity, AF.Rsqrt}
    orig = hw_specs.get_activation_tables

    def patched(arch):
        out = {}
        for name, funcs in orig(arch).items():
            out[name] = funcs if name == "natural_log_exp_and_others" \
                else (funcs - mine)
        return out

    bacc.get_activation_tables = patched
    _TABLES_PATCHED = True


def build_nc(T: int = 2048):
    """Build the single-core SPMD program for per-batch token count T."""
    NT = T // 128            # token tiles per batch
    NQC = T // 256           # query chunks per batch
    _patch_act_tables()
    nc = bacc.Bacc("TRN2", target_bir_lowering=False, debug=False)

    x8d = nc.dram_tensor("x8d", [D, B * T], FP8, kind="ExternalInput").ap()
    xrd = nc.dram_tensor("xrd", [D, B * T], FP8, kind="ExternalInput").ap()
    wd = {}
    for nm in ("wq8", "wqr", "wk8", "wkr", "wv8", "wvr"):
        wd[nm] = nc.dram_tensor(nm, [D, OF], FP8, kind="ExternalInput").ap()
    cosd = nc.dram_tensor("cosd", [128, NT * 128], F16, kind="ExternalInput").ap()
    sind = nc.dram_tensor("sind", [128, NT * 128], F16, kind="ExternalInput").ap()
    identd = nc.dram_tensor("identd", [128, 128], F16, kind="ExternalInput").ap()
    onesd = nc.dram_tensor("onesd", [128, 1], F16, kind="ExternalInput").ap()
    maskd = nc.dram_tensor("maskd", [128, 384], F32, kind="ExternalInput").ap()
    lamd = nc.dram_tensor("lamd", [128, 1], F32, kind="ExternalInput").ap()
    lamrowd = nc.dram_tensor("lamrowd", [1, 128], F16, kind="ExternalInput").ap()
    out = nc.dram_tensor("out", [B, T, OF], F16, kind="ExternalOutput").ap()

    # [128, (g two), t]: contraction chunk c = 2g+two, row p -> d = c*128+p
    x8_r = x8d.rearrange("(g two p) t -> p g two t", p=128, two=2)
    xr_r = xrd.rearrange("(g two p) t -> p g two t", p=128, two=2)
    w_r = {nm: ap.rearrange("(g two p) n -> p g two n", p=128, two=2)
           for nm, ap in wd.items()}

    with tile.TileContext(nc) as tc, ExitStack() as ctx:
        # --- constant pools (loaded once) ---
        cpool = ctx.enter_context(tc.tile_pool(name="consts", bufs=1))
        cos_t = cpool.tile([128, NT, 128], F16, tag="cos")
        sin_t = cpool.tile([128, NT, 128], F16, tag="sin")
        id_t = cpool.tile([128, 128], F16, tag="ident")
        ones_t = cpool.tile([128, 1], F16, tag="ones")
        lam_row = cpool.tile([1, 128], F16, tag="lamrow")
        mask_t = cpool.tile([128, 384], F32, tag="mask")
        eps_t = cpool.tile([128, 1], F32, tag="eps")
        ln16_t = cpool.tile([128, 1], F32, tag="ln16")
        pbias_t = cpool.tile([128, 1], F32, tag="pbias")
        nc.vector.memset(eps_t[:], EPS)
        nc.vector.memset(ln16_t[:], -math.log(16.0))
        nc.vector.memset(pbias_t[:], PBIAS)

        cos_r = cosd.rearrange("p (n f) -> p n f", f=128)
        sin_r = sind.rearrange("p (n f) -> p n f", f=128)

        def load_consts_early(nt0, nt1):
            # the first tiles' rotary tables + the small constants
            nc.sync.dma_start(cos_t[:, nt0:nt1], cos_r[:, nt0:nt1])
            nc.sync.dma_start(sin_t[:, nt0:nt1], sin_r[:, nt0:nt1])

        def load_consts_small():
            nc.sync.dma_start(id_t[:], identd)
            nc.sync.dma_start(ones_t[:], onesd)
            nc.sync.dma_start(mask_t[:], maskd)
            nc.sync.dma_start(lam_row[:], lamrowd)

        # --- weights: all six fp8 shards resident for the whole kernel ---
        wpool = ctx.enter_context(tc.tile_pool(name="weights", bufs=1))
        wt = {}
        for nm in ("wq8", "wqr", "wk8", "wkr", "wv8", "wvr"):
            wt[nm] = wpool.tile([128, NG, 2, OF], FP8, tag=nm, name=nm)

        # --- per-batch resident x (fp8 + residual) ---
        xpool = ctx.enter_context(tc.tile_pool(name="x", bufs=1))
        x8sb = xpool.tile([128, NG, 2, T], FP8, tag="x8sb", name="x8sb")
        xrsb = xpool.tile([128, NG, 2, T], FP8, tag="xrsb", name="xrsb")

        # --- per-batch key/value caches ---
        kvpool = ctx.enter_context(tc.tile_pool(name="kv", bufs=1))
        # --- streaming pools ---
        tpool = ctx.enter_context(tc.tile_pool(name="t", bufs=2))
        qtpool = ctx.enter_context(tc.tile_pool(name="qt", bufs=3))
        # pp tiles for the whole query chunk (both streams) stay live until
        # the ptilde merge: 2*(NQC pairs) + in-flight
        ppool = ctx.enter_context(tc.tile_pool(name="p", bufs=2 * NQC + 2))
        ptpool = ctx.enter_context(tc.tile_pool(name="pt", bufs=2))
        spool = ctx.enter_context(tc.tile_pool(name="stats", bufs=6))
        cbpool = ctx.enter_context(tc.tile_pool(name="cb", bufs=2))
        ypool = ctx.enter_context(tc.tile_pool(name="y", bufs=2))
        # --- PSUM pools: big 2 + out 2 + l 1 + (st|cb shared) 3 = 8 banks ---
        bigps = ctx.enter_context(tc.tile_pool(name="bigps", bufs=2, space="PSUM"))
        outps = ctx.enter_context(tc.tile_pool(name="outps", bufs=2, space="PSUM"))
        lps = ctx.enter_context(tc.tile_pool(name="lps", bufs=1, space="PSUM"))
        strps = ctx.enter_context(tc.tile_pool(name="strps", bufs=3, space="PSUM"))

        def proj(t0, chains):
            """PSUM [tok 128, OF] via DoubleRow accumulation chains.

            chains: list of (x_tile, w_tile); each contributes NG DR matmuls.
            """
            ps = bigps.tile([128, OF], F32, tag="big", name="projps")
            n = len(chains) * NG
            i = 0
            for xt, wtile in chains:
                for g in range(NG):
                    nc.tensor.matmul(
                        ps[:], xt[:, g, :, t0:t0 + 128], wtile[:, g],
                        start=(i == 0), stop=(i == n - 1),
                        perf_mode=mybir.MatmulPerfMode.DoubleRow)
                    i += 1
            return ps

        def qk_rot(ps, tt, is_q):
            """rms-stats + rotary + normalize [tok,512] psum -> yn f16 SBUF.

            ACT/DVE only (no PE). Rotary multiplies read the proj PSUM
            directly (f16 products); the bank frees after the two DVE mults.
            r = Rsqrt fuses the whole rms scale: for q,
            rsqrt(ss/256+eps)/16 == rsqrt(ss + 256 eps).
            """
            # rotary multiplies straight from PSUM, f16 out
            cos_b = cos_t[:, tt:tt + 1, :].broadcast_to([128, 4, 128])
            sin_b = sin_t[:, tt:tt + 1, :].broadcast_to([128, 4, 128])
            t1t = tpool.tile([128, OF], F16, tag="t1")
            t2t = tpool.tile([128, OF], F16, tag="t2")
            ps4 = ps.rearrange("p (a f) -> p a f", f=128)
            t1 = t1t.rearrange("p (a f) -> p a f", f=128)
            t2 = t2t.rearrange("p (a f) -> p a f", f=128)
            nc.vector.tensor_tensor(t1[:], ps4[:], cos_b, ALU.mult)
            nc.vector.tensor_tensor(t2[:], ps4[:], sin_b, ALU.mult)
            # rotary combine (f16, 2x DVE) then normalize
            ytt = tpool.tile([128, OF], F16, tag="yt")
            yv = ytt.rearrange("p (h u f) -> p h u f", h=2, f=128)
            t1v = t1t.rearrange("p (h u f) -> p h u f", h=2, f=128)
            t2v = t2t.rearrange("p (h u f) -> p h u f", h=2, f=128)
            nc.vector.tensor_tensor(yv[:, :, 0], t1v[:, :, 0], t2v[:, :, 1], ALU.add)
            nc.vector.tensor_tensor(yv[:, :, 1], t1v[:, :, 1], t2v[:, :, 0],
                                    ALU.subtract)
            # sum of squares per head from the ROTATED f16 (rotation is
            # norm-preserving): the proj PSUM bank frees right after the
            # two rotary mults instead of waiting on ACT's queue
            tsq = tpool.tile([128, OF], F16, tag="tsq")
            ss = spool.tile([128, 2], F32, tag="ss")
            for h in range(2):
                nc.scalar.activation(
                    tsq[:, h * 256:(h + 1) * 256],
                    ytt[:, h * 256:(h + 1) * 256],
                    AF.Square, accum_out=ss[:, h:h + 1])
            # r = rsqrt(mean + eps) = exp(-0.5*ln(mean+eps)); q also /16
            lg = spool.tile([128, 2], F32, tag="lg")
            nc.scalar.activation(lg[:], ss[:], AF.Ln, scale=1.0 / 256.0,
                                 bias=eps_t[:, 0:1])
            r = spool.tile([128, 2], F32, tag="r")
            nc.scalar.activation(r[:], lg[:], AF.Exp, scale=-0.5,
                                 bias=(ln16_t[:, 0:1] if is_q else 0.0))
            ynt = tpool.tile([128, OF], F16, tag="yn")
            yn = ynt[:]
            for h in range(2):
                nc.scalar.mul(yn[:, h * 256:(h + 1) * 256],
                              ytt[:, h * 256:(h + 1) * 256], r[:, h:h + 1])
            return ynt

        def do_transpose(yn, dst, dst_col):
            """DMA-transpose yn [tok,512] f16 into dst[:, :, dst_col:+128].

            dst[p, c, t] = yn[t, c*128 + p] (XBAR tile transpose) — runs on
            the DMA engines, so it never blocks the PE queue. Latency is
            hidden by the one-chunk-ahead pipelining.
            """
            nc.sync.dma_start_transpose(dst[:, :, dst_col:dst_col + 128],
                                        yn[:])

        def dma_x(c0, c1, bb, g=None):
            dsl = slice(c0, c1)
            ssl = slice(bb * T + c0, bb * T + c1)
            if g is None:
                nc.sync.dma_start(x8sb[:, :, :, dsl], x8_r[:, :, :, ssl])
                nc.sync.dma_start(xrsb[:, :, :, dsl], xr_r[:, :, :, ssl])
            else:
                nc.sync.dma_start(x8sb[:, g, :, dsl], x8_r[:, g, :, ssl])

        def dma_xr(c0, c1, bb, g):
            ssl = slice(bb * T + c0, bb * T + c1)
            nc.sync.dma_start(xrsb[:, g, :, c0:c1], xr_r[:, g, :, ssl])

        def dma_w(nm, h):
            hs = slice(h * 2, h * 2 + 2)
            nc.sync.dma_start(wt[nm][:, hs], w_r[nm][:, hs])

        for b in range(B):
            # DMA transfers serialize on the DMA engines, so the load order
            # is arranged to match PE consumption: k weights + the first x
            # tokens in fine slices, rotary tables for the first tiles,
            # the k-residual chain inputs, then v weights and the remaining
            # x in >=512B-line chunks (lines under 512B pay 2x DMA latency).
            if T < 2048:
                dma_x(0, T, b)
                if b == 0:
                    for nm in ("wk8", "wkr", "wv8", "wvr", "wq8", "wqr"):
                        nc.sync.dma_start(wt[nm][:], w_r[nm])
                    load_consts_early(0, NT)
                    load_consts_small()
            elif b == 0:
                for h in range(4):
                    dma_w("wk8", h)
                    dma_x(0, 256, b, g=2 * h)
                    dma_x(0, 256, b, g=2 * h + 1)
                load_consts_early(0, 4)
                load_consts_small()
                for h in range(4):
                    dma_w("wkr", h)
                    dma_xr(0, 256, b, 2 * h)
                    dma_xr(0, 256, b, 2 * h + 1)
                dma_x(256, 768, b)
                for h in range(4):
                    dma_w("wv8", h)
                dma_x(768, 1280, b)
                for h in range(4):
                    dma_w("wvr", h)
                load_consts_early(4, NT)
                dma_x(1280, 1792, b)
                nc.sync.dma_start(wt["wq8"][:], w_r["wq8"])
                dma_x(1792, T, b)
                nc.sync.dma_start(wt["wqr"][:], w_r["wqr"])
            else:
                # batch 0's query chunks run high-to-low, so its readers of
                # the HIGH token range finish first: reload those first
                for c0 in range(T - 512, -1, -512):
                    dma_x(c0, c0 + 512, b)

            kT = kvpool.tile([128, 4, T], F16, tag="kT")
            vsb = kvpool.tile([128, NT, OF], F16, tag="v")

            # --- Phase A: keys and values (kT built by DMA transpose).
            # v-projections lag the k-projections by VLAG tiles: the v
            # weights arrive later in the startup DMA stream, and the
            # v-only tail is a convenient PE-heavy window to hide the
            # phase-B q-chunk priming's DVE work under. ---
            def do_k(tt):
                kps = proj(tt * 128, [(x8sb, wt["wk8"]), (xrsb, wt["wk8"]),
                                      (x8sb, wt["wkr"])])
                yn = qk_rot(kps, tt, False)
                do_transpose(yn, kT, tt * 128)

            def do_v(tt):
                vps = proj(tt * 128, [(x8sb, wt["wv8"]), (xrsb, wt["wv8"]),
                                      (x8sb, wt["wvr"])])
                nc.scalar.mul(vsb[:, tt], vps[:], 1.0 / WS)

            # v lags k by VLAG tiles: the v weights arrive after the k
            # weights and first x chunks in the serialized DMA stream
            VLAG = min(4, NT // 2)
            # batch 1 builds kT/vsb high-to-low, matching its reversed x
            # reload order (and its query chunks also run high-to-low)
            torder = list(range(NT)) if b == 0 else list(range(NT - 1, -1, -1))
            aseq = []
            for i_t, tt in enumerate(torder):
                aseq.append(('k', tt))
                if i_t >= VLAG:
                    aseq.append(('v', torder[i_t - VLAG]))
            for tt in torder[NT - VLAG:]:
                aseq.append(('v', tt))

            # --- Phase B: queries + attention, q-chunks prefetched TWO qc
            # ahead. The prefetched chunk's PE projections are emitted as
            # single-matmul thunks and drained as filler wherever PE would
            # otherwise wait (between score pairs while exp catches up, and
            # under the DVE cb-chain); its DVE rotary is emitted after the
            # ptilde merges so the merges aren't stuck behind it in the
            # in-order DVE queue.
            def q_proj_thunks(qc):
                """Allocate PSUM + return (ps_tiles, [thunk emitting 1 mm])."""
                pss = []
                thunks = []
                for u in range(2):
                    ps = bigps.tile([128, OF], F32, tag="big",
                                    name=f"qps{qc}_{u}")
                    pss.append(ps)
                    t0 = (qc * 2 + u) * 128
                    chain = [(x8sb, wt["wq8"]), (xrsb, wt["wq8"]),
                             (x8sb, wt["wqr"])]
                    n = len(chain) * NG

                    def mk(ps_, t0_, xt_, wtile_, g_, i_):
                        def emit():
                            nc.tensor.matmul(
                                ps_[:], xt_[:, g_, :, t0_:t0_ + 128],
                                wtile_[:, g_],
                                start=(i_ == 0), stop=(i_ == n - 1),
                                perf_mode=mybir.MatmulPerfMode.DoubleRow)
                        return emit
                    i = 0
                    for xt, wtile in chain:
                        for g in range(NG):
                            thunks.append(mk(ps, t0, xt, wtile, g, i))
                            i += 1
                return pss, thunks

            def q_rot_part(qc, qpss):
                qT = qtpool.tile([128, 4, 256], F16, tag="qT", name=f"qT{qc}")
                for u in range(2):
                    yn = qk_rot(qpss[u], qc * 2 + u, True)
                    do_transpose(yn, qT, u * 128)
                return qT

            # prime chunks 0 and 1 (their rotary DVE work hides under the
            # v-projection tail of phase A)
            def q_chunk_now(qc):
                pss, thunks = q_proj_thunks(qc)
                for th in thunks:
                    th()
                return q_rot_part(qc, pss)

            # batch 1's query chunks run in REVERSE order: the trailing,
            # unoverlappable p@v at the very end of the kernel is then the
            # 1-pair qc=0 instead of the full-width last chunk
            qorder = list(range(NQC))
            if NQC > 1:
                qorder = qorder[::-1]

            qT_cur = qT_next = None
            tail = max(len(aseq) - 4, 0)
            for idx, (kind, tt) in enumerate(aseq):
                if idx == tail:
                    qT_cur = q_chunk_now(qorder[0])
                if idx == tail + 2 and NQC > 1:
                    qT_next = q_chunk_now(qorder[1])
                (do_k if kind == 'k' else do_v)(tt)
            if qT_cur is None:
                qT_cur = q_chunk_now(qorder[0])
            if qT_next is None and NQC > 1:
                qT_next = q_chunk_now(qorder[1])
            # Two-stage software pipeline over query chunks: iteration qc
            # emits S(qc) (scores+exp+l), the cb chain and ptilde merges for
            # qc, but the p@v matmuls for qc-1 — interleaved into S(qc) as
            # PE filler. The ptilde tensors of qc-1 were finished while
            # S(qc-1)'s tail ran, so pv(qc-1) is never blocked, and the cb
            # chain of qc hides under pv(qc-1)'s leftovers + projections.
            prev = None          # (pts, linv, qc) of the previous iteration

            def emit_pv(state, op_, j):
                pts_, _linv, qcp = state
                nktp = 2 * (qcp + 1)
                pt = pts_[j]
                for sl in range(2):
                    kt = 2 * j + sl
                    for u in range(2):
                        if kt == nktp - 1 and u == 0:
                            continue
                        lastu = nktp - 1 if u == 1 else nktp - 2
                        nc.tensor.matmul(
                            op_[u][:], pt[:, sl, u * 128:(u + 1) * 128],
                            vsb[:, kt], start=(kt == 0), stop=(kt == lastu))

            def emit_final(state, op_):
                _pts, linv_, qcp = state
                for u in range(2):
                    yf = ypool.tile([128, OF], F16, tag="yf")
                    nc.scalar.mul(yf[:], op_[u][:], linv_[:, 0, u:u + 1])
                    t0 = qcp * 256 + u * 128
                    nc.sync.dma_start(out[b, t0:t0 + 128, :], yf[:])

            for i_qc in range(NQC):
                qc = qorder[i_qc]
                qT = qT_cur

                # the chunk prefetched two ahead: its projection matmuls are
                # drained as PE filler through this iteration
                if i_qc + 2 < NQC:
                    pf_pss, pf_thunks = q_proj_thunks(qorder[i_qc + 2])
                else:
                    pf_pss, pf_thunks = None, []

                op = [outps.tile([128, OF], F32, tag="o", name=f"op{u_}")
                      for u_ in range(2)] if prev is not None else None
                pvq = list(range(prev[2] + 1)) if prev is not None else []

                def filler(k, tail=False):
                    for _ in range(k):
                        if pvq:
                            emit_pv(prev, op, pvq.pop(0))
                        elif pf_thunks:
                            pf_thunks.pop(0)()

                nkt = 2 * (qc + 1)
                npair = nkt // 2
                # l accumulators for both streams / both query halves
                lp = lps.tile([128, 2, 2, NT], F32, tag="l")
                pps = [[None] * npair, [None] * npair]

                def emit_pair(s, j):
                    # two key tiles share one PSUM bank: slot 0's
                    # start=True lazily zeroes the whole bank, slot 1
                    # accumulates onto the pending zeros.
                    stp = strps.tile([128, 2, 256], F32, tag="str",
                                     name="stpair")
                    i = 0
                    for sl in range(2):
                        kt = 2 * j + sl
                        qcols = slice(128, 256) if kt == nkt - 1 \
                            else slice(0, 256)
                        for c2 in range(2):
                            nc.tensor.matmul(
                                stp[:, sl, qcols],
                                kT[:, 2 * s + c2, kt * 128:(kt + 1) * 128],
                                qT[:, 2 * s + c2, qcols],
                                start=(i == 0), stop=(i == 3),
                                skip_group_check=True)
                            i += 1
                    if j == qc:
                        # diagonal pair: triangular mask per u-half
                        for sl in range(2):
                            mc = slice(0, 128) if sl == 0 \
                                else slice(128, 256)
                            nc.vector.tensor_tensor(
                                stp[:, sl, mc], stp[:, sl, mc],
                                mask_t[:, 128:256], ALU.add)
                    return stp

                def emit_exp(s, j, stp):
                    pp = ppool.tile([128, 2, 256], F16, tag="p")
                    with tc.high_priority(offset=100):
                        if j == qc:   # diag pair: slot 1 only u1 cols
                            nc.scalar.activation(pp[:, 0], stp[:, 0],
                                                 AF.Exp,
                                                 bias=pbias_t[:, 0:1])
                            nc.scalar.activation(pp[:, 1, 128:256],
                                                 stp[:, 1, 128:256],
                                                 AF.Exp,
                                                 bias=pbias_t[:, 0:1])
                        else:
                            nc.scalar.activation(pp[:], stp[:], AF.Exp,
                                                 bias=pbias_t[:, 0:1])
                    return pp

                # --- S phase: scores + exp + l for both streams, with
                # prefetch-projection matmuls as filler (exp throughput is
                # the S-phase bottleneck; the filler soaks PE's wait).
                # Stream 0's l reduction is emitted mid-phase so it hides
                # under stream 1's score matmuls. ---
                lsum = spool.tile([128, 2, 2], F32, tag="lsum")
                linv = spool.tile([128, 2, 2], F32, tag="linv")
                jorder = list(range(npair))
                for s in range(2):
                    pairq = [(j_, emit_pair(s, j_))
                             for j_ in jorder[:min(2, npair)]]
                    for i_ in range(npair):
                        j, stp = pairq.pop(0)
                        pp = emit_exp(s, j, stp)
                        pps[s][j] = pp
                        if i_ + 2 < npair:
                            j2 = jorder[i_ + 2]
                            pairq.append((j2, emit_pair(s, j2)))
                        filler(1)
                        for sl in range(2):
                            kt = 2 * j + sl
                            for u in range(2):
                                if kt == nkt - 1 and u == 0:
                                    continue
                                nc.tensor.matmul(
                                    lp[:, s, u, kt:kt + 1],
                                    pp[:, sl, u * 128:(u + 1) * 128],
                                    ones_t[:], start=True, stop=True)
                    nc.vector.reduce_sum(lsum[:, s, 0:1],
                                         lp[:, s, 0, 0:max(nkt - 1, 1)],
                                         axis=mybir.AxisListType.X)
                    nc.vector.reduce_sum(lsum[:, s, 1:2],
                                         lp[:, s, 1, 0:nkt],
                                         axis=mybir.AxisListType.X)
                    nc.vector.reciprocal(linv[:, s], lsum[:, s])
                # cb2 = l1/l2 (lam is folded into the broadcast matmul)
                cb16 = spool.tile([128, 2], F16, tag="cb16")
                nc.vector.tensor_tensor(cb16[:], lsum[:, 0], linv[:, 1],
                                        ALU.mult)

                # drain remaining pv chunks + projections over the serial
                # lsum/cb chain latency
                filler(len(pvq) + len(pf_thunks))

                # transpose each cb16 column to its own partition-0 row
                # (engines can only address partitions 0/32/64)
                cbT = [strps.tile([1, 128], F16, tag="str", name=f"cbT{u_}")
                       for u_ in range(2)]
                for u in range(2):
                    nc.tensor.transpose(cbT[u][:], cb16[:, u:u + 1], id_t[:])
                cbrow = [cbpool.tile([1, 128], F16, tag=f"cbrow{u_}",
                                     name=f"cbrow{u_}")
                         for u_ in range(2)]
                for u in range(2):
                    nc.vector.tensor_copy(cbrow[u][:], cbT[u][:])
                cbbps = strps.tile([128, 2, 128], F32, tag="str", name="cbb")
                for u in range(2):
                    nc.tensor.matmul(cbbps[:, u], lam_row[:], cbrow[u][:],
                                     start=True, stop=True,
                                     skip_group_check=True)
                cbb = cbpool.tile([128, 1, 256], F16, tag="cbbs")
                nc.vector.tensor_copy(cbb[:, 0], cbbps[:])
                cbb_b = cbb[:].broadcast_to([128, 2, 256])

                # --- ptilde merges for THIS qc (consumed next iteration) ---
                pts = []
                cbbps_b = cbbps[:].rearrange("p u f -> p (u f)") \
                    .rearrange("p (one f) -> p one f", one=1) \
                    .broadcast_to([128, 2, 256])
                for j in range(npair):
                    tmp = ptpool.tile([128, 2, 256], F16, tag="pt", bufs=2)
                    # the first merge reads the broadcast row straight from
                    # PSUM (slower op, but skips waiting for the SBUF copy)
                    nc.vector.tensor_tensor(tmp[:], pps[1][j][:],
                                            cbbps_b if j == 0 else cbb_b,
                                            ALU.mult)
                    pt = ptpool.tile([128, 2, 256], F16, tag="pt2",
                                     bufs=NQC + 2)
                    nc.vector.tensor_tensor(pt[:], pps[0][j][:], tmp[:],
                                            ALU.subtract)
                    pts.append(pt)

                # drain the rest of pv(qc-1) and close it out
                filler(len(pvq) + len(pf_thunks))
                if prev is not None:
                    emit_final(prev, op)

                # the prefetched chunk's DVE rotary + transposes, behind
                # the ptilde merges in the DVE queue
                qT_cur = qT_next
                if pf_pss is not None:
                    qT_next = q_rot_part(qorder[i_qc + 2], pf_pss)

                prev = (pts, linv, qc)

            # trailing p@v for the last query chunk of this batch
            op = [outps.tile([128, OF], F32, tag="o", name=f"opt{u_}")
                  for u_ in range(2)]
            for j in range(prev[2] + 1):
                emit_pv(prev, op, j)
            emit_final(prev, op)
    nc.compile()
    return nc


def make_in_maps(x, Wq, Wk, Wv, lam, T):
    """Host-side sharding + fp8/f16 layout prep. Returns list of 8 in_maps."""
    NT = T // 128
    xf = np.ascontiguousarray(x.reshape(B * T, D).T).astype(np.float32)
    x8 = q8(xf)
    xr = q8(xf - x8.astype(np.float32))
    t = np.arange(T, dtype=np.float64)
    inv = 1.0 / (10000.0 ** (np.arange(0, HEAD_DIM, 2, dtype=np.float64)
                             / HEAD_DIM))
    fr = np.outer(t, inv)                                    # [T, 128]
    cos = np.cos(fr).astype(np.float32)
    sin = np.sin(fr).astype(np.float32)
    # [128, NT*128]: row p, col tt*128+f  ->  cos[tt*128+p, f]
    cos_sb = np.ascontiguousarray(
        cos.reshape(NT, 128, 128).transpose(1, 0, 2).reshape(128, NT * 128)
    ).astype(np.float16)
    sin_sb = np.ascontiguousarray(
        sin.reshape(NT, 128, 128).transpose(1, 0, 2).reshape(128, NT * 128)
    ).astype(np.float16)
    ident = np.eye(128, dtype=np.float16)
    ones1 = np.ones((128, 1), np.float16)
    ii = np.arange(128).reshape(128, 1)
    mm_ = np.arange(384).reshape(1, 384) - 128
    maskneg = np.where(mm_ >= ii, 0.0, NEG).astype(np.float32)
    lam_np = np.full((128, 1), lam, np.float32)
    lam_row_np = np.full((1, 128), lam, np.float16)

    common = {"x8d": x8, "xrd": xr, "cosd": cos_sb, "sind": sin_sb,
              "identd": ident, "onesd": ones1, "maskd": maskneg,
              "lamd": lam_np, "lamrowd": lam_row_np}
    in_maps = []
    for i in range(8):
        def shards(W, half):
            sh = np.concatenate(
                [W[i * 256:(i + 1) * 256], W[(i + 8) * 256:(i + 9) * 256]], 0
            ) if half else W[i * 512:(i + 1) * 512]
            wT = np.ascontiguousarray(sh.T).astype(np.float32) * WS
            w8 = q8(wT)
            wr = q8(wT - w8.astype(np.float32))
            return w8, wr
        m = dict(common)
        m["wq8"], m["wqr"] = shards(np.asarray(Wq), True)
        m["wk8"], m["wkr"] = shards(np.asarray(Wk), True)
        m["wv8"], m["wvr"] = shards(np.asarray(Wv), False)
        in_maps.append(m)
    return in_maps


_NC_CACHE: dict = {}


def run_cores(x, Wq, Wk, Wv, lambda_q1, lambda_k1, lambda_q2, lambda_k2,
              T=2048, **spmd_kwargs):
    lam1 = np.exp(np.float32(np.dot(lambda_q1.astype(np.float32),
                                    lambda_k1.astype(np.float32))))
    lam2 = np.exp(np.float32(np.dot(lambda_q2.astype(np.float32),
                                    lambda_k2.astype(np.float32))))
    lam = np.float32(lam1 - lam2 + np.float32(LAMBDA_INIT))
    if T not in _NC_CACHE:
        _NC_CACHE[T] = build_nc(T)
    nc = _NC_CACHE[T]
    in_maps = make_in_maps(np.asarray(x), np.asarray(Wq), np.asarray(Wk),
                           np.asarray(Wv), lam, T)
    res = run_bass_kernel_spmd(nc, in_maps, core_ids=list(range(8)),
                               **spmd_kwargs)
    shards = [res.results[i]["out"] for i in range(8)]       # [B,T,512] each
    y = np.stack(shards, axis=2).reshape(B, T, N_HEADS * HEAD_DIM)
    return y, res


def kernel(x, Wq, Wk, Wv, lambda_q1, lambda_k1, lambda_q2, lambda_k2):
    y, _ = run_cores(x, Wq, Wk, Wv, lambda_q1, lambda_k1, lambda_q2,
                     lambda_k2, T=x.shape[1])
    return y.astype(np.float32)
